# revision 8
# baseline (speedup 1.0000x reference)
"""Multi-head attention (N=2, K=2048, DIN=1024, H=16, DOUT=64) on 8 TRN2 NeuronCores.

Sharding: data-parallel over batch N (cores 0-3 -> n=0, cores 4-7 -> n=1),
tensor-parallel over heads (4 heads per core). Each core computes its 4 heads'
attention plus the partial output projection over its head-feature slice of Wp;
the host sums the 4 partials per batch element and adds the output bias.

Per-core kernel structure (all matmuls bf16, fp32 PSUM accumulation):
  - host pre-transposes/casts activations to bf16 [DIN, SEQ] so projection
    matmuls contract over DIN on partitions with natural contiguous DMA.
  - q/k projected head-pair-wise to [dout(2 heads on partitions), seq];
    v projected to natural [seq, 4*64] layout with a ones column appended per
    head (softmax denominator rides along the PV matmul for free).
  - scores computed transposed (S.T = k_h @ q_h.T: seq_k on partitions), so
    softmax probabilities are already in the layout the PV matmul needs.
  - no max-subtraction: scores are bounded (~|S/8| < 10), exp is safe in fp32.
  - mask applied as a 0/1 multiply AFTER exp (identical to -1e9 pre-masking).
  - normalization: denominator from the ones column, reciprocal on DVE,
    broadcast across partitions via a K=1 outer-product matmul.
"""

import numpy as np
import ml_dtypes

import concourse.bass as bass
import concourse.mybir as mybir
from concourse import bacc
from concourse.tile import TileContext

P = 128
SEQ = 2048
DIN = 1024
DOUT = 64
H = 16
N = 2
HPC = 4  # heads per core
NPAIR = 2  # head pairs per core
KSUB = DIN // P  # 8 contraction subtiles for projections
NKT = SEQ // P  # 16 seq_k tiles of 128
VW = DOUT + 1  # v columns per head incl. ones column
BF = mybir.dt.bfloat16
F32 = mybir.dt.float32
BF_NP = ml_dtypes.bfloat16

_NC_CACHE = None


def build_bass():
    nc = bacc.Bacc()

    xq_d = nc.declare_dram_parameter("xqT", [DIN, SEQ], BF, isOutput=False)
    xk_d = nc.declare_dram_parameter("xkT", [DIN, SEQ], BF, isOutput=False)
    xv_d = nc.declare_dram_parameter("xvT", [DIN, SEQ], BF, isOutput=False)
    mk_d = nc.declare_dram_parameter("maskT", [SEQ, SEQ], BF, isOutput=False)
    wq_d = nc.declare_dram_parameter("wq", [DIN, HPC * DOUT], BF, isOutput=False)
    wk_d = nc.declare_dram_parameter("wk", [DIN, HPC * DOUT], BF, isOutput=False)
    wv_d = nc.declare_dram_parameter("wv", [DIN, HPC * DOUT], BF, isOutput=False)
    wp_d = nc.declare_dram_parameter("wp", [HPC * DOUT, DIN], BF, isOutput=False)
    bqp_d = nc.declare_dram_parameter("bqp", [P, NPAIR], F32, isOutput=False)
    bkp_d = nc.declare_dram_parameter("bkp", [P, NPAIR], F32, isOutput=False)
    bvr_d = nc.declare_dram_parameter("bvr", [P, HPC * DOUT], F32, isOutput=False)
    out_d = nc.declare_dram_parameter("out", [SEQ, DIN], F32, isOutput=True)

    ADD = mybir.AluOpType.add
    EXP = mybir.ActivationFunctionType.Exp

    with TileContext(nc) as tc:
        with (
            tc.tile_pool(name="const", bufs=1) as const,
            tc.tile_pool(name="xin", bufs=1) as xin,
            tc.tile_pool(name="proj", bufs=1) as proj,
            tc.tile_pool(name="maskp", bufs=3) as maskp,
            tc.tile_pool(name="ptp", bufs=3) as ptp,
            tc.tile_pool(name="epi", bufs=4) as epi,
            tc.tile_pool(name="ps_s", bufs=2, space="PSUM") as ps_s,
            tc.tile_pool(name="ps_sm", bufs=4, space="PSUM") as ps_sm,
        ):
            # ---- constants -------------------------------------------------
            wq_sb = const.tile([P, KSUB, HPC * DOUT], BF)
            nc.sync.dma_start(wq_sb[:], wq_d.rearrange("(o p) m -> p o m", p=P))
            wk_sb = const.tile([P, KSUB, HPC * DOUT], BF)
            nc.sync.dma_start(wk_sb[:], wk_d.rearrange("(o p) m -> p o m", p=P))
            wv_sb = const.tile([P, KSUB, HPC * DOUT], BF)
            nc.sync.dma_start(wv_sb[:], wv_d.rearrange("(o p) m -> p o m", p=P))
            wp_sb = const.tile([P, NPAIR, DIN], BF)
            nc.sync.dma_start(wp_sb[:], wp_d.rearrange("(o p) n -> p o n", p=P))
            bqp_sb = const.tile([P, NPAIR], F32)
            nc.sync.dma_start(bqp_sb[:], bqp_d[:])
            bkp_sb = const.tile([P, NPAIR], F32)
            nc.sync.dma_start(bkp_sb[:], bkp_d[:])
            bvr_sb = const.tile([P, HPC * DOUT], F32)
            nc.sync.dma_start(bvr_sb[:], bvr_d[:])
            ones_sb = const.tile([1, DOUT], F32)
            nc.vector.memset(ones_sb[:], 1.0)

            # ---- resident transposed activations ---------------------------
            xq_sb = xin.tile([P, KSUB, SEQ], BF)
            nc.sync.dma_start(xq_sb[:], xq_d.rearrange("(o p) s -> p o s", p=P))
            xk_sb = xin.tile([P, KSUB, SEQ], BF)
            nc.sync.dma_start(xk_sb[:], xk_d.rearrange("(o p) s -> p o s", p=P))
            xv_sb = xin.tile([P, KSUB, SEQ], BF)
            nc.sync.dma_start(xv_sb[:], xv_d.rearrange("(o p) s -> p o s", p=P))

            # ---- persistent intermediates ----------------------------------
            qhT = proj.tile([P, NPAIR, SEQ], BF)  # [2-head dout, pair, seq]
            khT = proj.tile([P, NPAIR, SEQ], BF)
            vsb = proj.tile([P, NKT, HPC * VW], BF)  # v + ones col per head
            ynT = proj.tile([P, NPAIR, SEQ], BF)  # normalized y.T

            vsb4 = vsb.rearrange("p k (h c) -> p k h c", c=VW)
            for h in range(HPC):
                nc.vector.memset(vsb4[:, :, h, DOUT : DOUT + 1], 1.0)

            # ---- phase A: projections --------------------------------------
            # q/k head-pair-wise: psum[2*64 dout, 512 seq]
            for pair in range(NPAIR):
                for w_sb, x_sb, b_sb, o_sb in (
                    (wq_sb, xq_sb, bqp_sb, qhT),
                    (wk_sb, xk_sb, bkp_sb, khT),
                ):
                    for qt in range(SEQ // 512):
                        pps = ps_sm.tile([P, 512], F32, tag="sm", name="pps")
                        for o in range(KSUB):
                            nc.tensor.matmul(
                                pps[:],
                                w_sb[:, o, pair * P : (pair + 1) * P],
                                x_sb[:, o, qt * 512 : (qt + 1) * 512],
                                start=(o == 0),
                                stop=(o == KSUB - 1),
                            )
                        nc.vector.tensor_tensor(
                            o_sb[:, pair, qt * 512 : (qt + 1) * 512],
                            pps[:],
                            b_sb[:, pair : pair + 1].to_broadcast((P, 512)),
                            ADD,
                        )
            # v natural layout: psum[128 seq, 256 dout]
            for st in range(NKT):
                vps = ps_sm.tile([P, 512], F32, tag="sm", name="vps")
                for o in range(KSUB):
                    nc.tensor.matmul(
                        vps[:, : HPC * DOUT],
                        xv_sb[:, o, st * P : (st + 1) * P],
                        wv_sb[:, o, :],
                        start=(o == 0),
                        stop=(o == KSUB - 1),
                    )
                nc.vector.tensor_tensor(
                    vsb4[:, st, :, 0:DOUT],
                    vps[:, : HPC * DOUT].rearrange("p (h c) -> p h c", c=DOUT),
                    bvr_sb.rearrange("p (h c) -> p h c", c=DOUT),
                    ADD,
                )

            # ---- phase B: attention ----------------------------------------
            for pair in range(NPAIR):
                for qh in range(2):  # halves of the seq_q axis (1024 each)
                    pvs = []
                    for i in range(4):
                        pv = ps_sm.tile([P, 512], F32, tag="sm", name=f"pv{i}")
                        pvs.append(pv)
                    for kt in range(NKT):
                        mt = maskp.tile([P, 1024], BF, tag="mt", name="mt")
                        nc.sync.dma_start(
                            mt[:],
                            mk_d[kt * P : (kt + 1) * P, qh * 1024 : (qh + 1) * 1024],
                        )
                        for h2 in range(2):
                            head = pair * 2 + h2
                            hs = slice(h2 * DOUT, (h2 + 1) * DOUT)
                            sps = ps_s.tile([P, 1024], F32, tag="s", name="sps")
                            for qi in range(2):
                                q0 = (qh * 2 + qi) * 512
                                nc.tensor.matmul(
                                    sps[:, qi * 512 : (qi + 1) * 512],
                                    khT[hs, pair, kt * P : (kt + 1) * P],
                                    qhT[hs, pair, q0 : q0 + 512],
                                    start=True,
                                    stop=True,
                                )
                            pt = ptp.tile([P, 1024], BF, tag="pt", name="pt")
                            nc.scalar.activation(pt[:], sps[:], EXP, scale=0.125)
                            nc.vector.tensor_mul(pt[:], pt[:], mt[:])
                            for qi in range(2):
                                nc.tensor.matmul(
                                    pvs[h2 * 2 + qi][: DOUT + 1, :],
                                    vsb4[:, kt, head, :],
                                    pt[:, qi * 512 : (qi + 1) * 512],
                                    start=(kt == 0),
                                    stop=(kt == NKT - 1),
                                )
                    # epilogue: normalize this half's outputs
                    for h2 in range(2):
                        for qi in range(2):
                            pv = pvs[h2 * 2 + qi]
                            q0 = (qh * 2 + qi) * 512
                            rd = epi.tile([1, 512], F32, tag="rd", name="rd")
                            nc.vector.reciprocal(rd[:], pv[DOUT : DOUT + 1, :])
                            ops = ps_s.tile([DOUT, 512], F32, tag="s", name="ops")
                            nc.tensor.matmul(
                                ops[:], ones_sb[:], rd[:], start=True, stop=True
                            )
                            rdb = epi.tile([DOUT, 512], BF, tag="rdb", name="rdb")
                            nc.vector.tensor_copy(rdb[:], ops[:])
                            nc.vector.tensor_mul(
                                ynT[h2 * DOUT : (h2 + 1) * DOUT, pair, q0 : q0 + 512],
                                pv[:DOUT, :],
                                rdb[:],
                            )

            # ---- phase C: output projection (partial over 4 heads) ---------
            for st in range(NKT):
                for ntile in range(DIN // 512):
                    cps = ps_sm.tile([P, 512], F32, tag="sm", name="cps")
                    for pair in range(NPAIR):
                        nc.tensor.matmul(
                            cps[:],
                            ynT[:, pair, st * P : (st + 1) * P],
                            wp_sb[:, pair, ntile * 512 : (ntile + 1) * 512],
                            start=(pair == 0),
                            stop=(pair == NPAIR - 1),
                        )
                    ost = epi.tile([P, 512], F32, tag="ost", name="ost", bufs=3)
                    nc.scalar.copy(ost[:], cps[:])
                    nc.sync.dma_start(
                        out_d[st * P : (st + 1) * P, ntile * 512 : (ntile + 1) * 512],
                        ost[:],
                    )

    nc.finalize()
    return nc


def make_in_maps(query, key, value, mask, Wq, bq, Wk, bk, Wv, bv, Wp, bp):
    """Shard + pre-layout the full inputs into 8 per-core input dicts."""
    in_maps = []
    for c in range(8):
        n = c // 4
        h0 = HPC * (c % 4)
        hs = slice(h0, h0 + HPC)

        def t_bf(x):  # [SEQ, DIN] -> contiguous [DIN, SEQ] bf16
            return np.ascontiguousarray(x.T).astype(BF_NP)

        # (H', DIN, DOUT) -> (DIN, H'*DOUT), head-major columns
        def w_bf(W):
            return np.ascontiguousarray(
                W[hs].transpose(1, 0, 2).reshape(DIN, HPC * DOUT)
            ).astype(BF_NP)

        # per-pair per-partition bias: [128, 2], col p = concat of heads (2p, 2p+1)
        def b_pair(b):
            return np.ascontiguousarray(b[hs].reshape(NPAIR, P).T).astype(np.float32)

        in_maps.append(
            {
                "xqT": t_bf(query[n]),
                "xkT": t_bf(key[n]),
                "xvT": t_bf(value[n]),
                "maskT": np.ascontiguousarray((~mask[n]).T).astype(BF_NP),
                "wq": w_bf(Wq),
                "wk": w_bf(Wk),
                "wv": w_bf(Wv),
                "wp": np.ascontiguousarray(
                    Wp[h0 * DOUT : (h0 + HPC) * DOUT, :]
                ).astype(BF_NP),
                "bqp": b_pair(bq),
                "bkp": b_pair(bk),
                "bvr": np.ascontiguousarray(
                    np.tile(bv[hs].reshape(1, HPC * DOUT), (P, 1))
                ).astype(np.float32),
            }
        )
    return in_maps


def kernel(**inputs):
    global _NC_CACHE
    from concourse.bass_utils import run_bass_kernel_spmd

    if _NC_CACHE is None:
        _NC_CACHE = build_bass()
    nc = _NC_CACHE

    in_maps = make_in_maps(**inputs)
    res = run_bass_kernel_spmd(nc, in_maps, core_ids=list(range(8))).results
    parts = [res[c]["out"].astype(np.float32) for c in range(8)]
    bp = inputs["bp"]
    out = np.stack(
        [
            parts[0] + parts[1] + parts[2] + parts[3] + bp[None, :],
            parts[4] + parts[5] + parts[6] + parts[7] + bp[None, :],
        ]
    )
    return out.astype(np.float32)


# revision 12
# speedup vs baseline: 1.0248x; 1.0248x over previous
"""Multi-head attention (N=2, K=2048, DIN=1024, H=16, DOUT=64) on 8 TRN2 NeuronCores.

Sharding: data-parallel over batch N (cores 0-3 -> n=0, cores 4-7 -> n=1),
tensor-parallel over heads (4 heads per core). Each core computes its 4 heads'
attention plus the partial output projection over its head-feature slice of Wp;
the host sums the 4 partials per batch element and adds the output bias.

Per-core kernel structure (all matmuls bf16, fp32 PSUM accumulation):
  - host pre-transposes/casts activations to bf16 [DIN, SEQ] so projection
    matmuls contract over DIN on partitions with natural contiguous DMA.
  - q/k projected head-pair-wise to [dout(2 heads on partitions), seq];
    v projected to natural [seq, 4*64] layout with a ones column appended per
    head (softmax denominator rides along the PV matmul for free).
  - scores computed transposed (S.T = k_h @ q_h.T: seq_k on partitions), so
    softmax probabilities are already in the layout the PV matmul needs.
  - no max-subtraction: scores are bounded (~|S/8| < 10), exp is safe in fp32.
  - mask applied as a 0/1 multiply AFTER exp (identical to -1e9 pre-masking).
  - normalization: denominator from the ones column, reciprocal on DVE,
    broadcast across partitions via a K=1 outer-product matmul.
"""

import numpy as np
import ml_dtypes

import concourse.bass as bass
import concourse.mybir as mybir
from concourse import bacc
from concourse.tile import TileContext

P = 128
SEQ = 2048
DIN = 1024
DOUT = 64
H = 16
N = 2
HPC = 4  # heads per core
NPAIR = 2  # head pairs per core
KSUB = DIN // P  # 8 contraction subtiles for projections
NKT = SEQ // P  # 16 seq_k tiles of 128
VW = DOUT + 1  # v columns per head incl. ones column
BF = mybir.dt.bfloat16
F32 = mybir.dt.float32
BF_NP = ml_dtypes.bfloat16

_NC_CACHE = None


def build_bass():
    nc = bacc.Bacc()

    xq_d = nc.declare_dram_parameter("xqT", [DIN, SEQ], BF, isOutput=False)
    xk_d = nc.declare_dram_parameter("xkT", [DIN, SEQ], BF, isOutput=False)
    xv_d = nc.declare_dram_parameter("xvT", [DIN, SEQ], BF, isOutput=False)
    mk_d = nc.declare_dram_parameter("maskT", [SEQ, SEQ], BF, isOutput=False)
    wq_d = nc.declare_dram_parameter("wq", [DIN, HPC * DOUT], BF, isOutput=False)
    wk_d = nc.declare_dram_parameter("wk", [DIN, HPC * DOUT], BF, isOutput=False)
    wv_d = nc.declare_dram_parameter("wv", [DIN, HPC * DOUT], BF, isOutput=False)
    wp_d = nc.declare_dram_parameter("wp", [HPC * DOUT, DIN], BF, isOutput=False)
    bqp_d = nc.declare_dram_parameter("bqp", [P, NPAIR], F32, isOutput=False)
    bkp_d = nc.declare_dram_parameter("bkp", [P, NPAIR], F32, isOutput=False)
    bvr_d = nc.declare_dram_parameter("bvr", [P, HPC * DOUT], F32, isOutput=False)
    out_d = nc.declare_dram_parameter("out", [SEQ, DIN], F32, isOutput=True)

    ADD = mybir.AluOpType.add
    EXP = mybir.ActivationFunctionType.Exp

    with TileContext(nc) as tc:
        with (
            tc.tile_pool(name="const", bufs=1) as const,
            tc.tile_pool(name="xin", bufs=1) as xin,
            tc.tile_pool(name="proj", bufs=1) as proj,
            tc.tile_pool(name="maskp", bufs=3) as maskp,
            tc.tile_pool(name="ptp", bufs=3) as ptp,
            tc.tile_pool(name="epi", bufs=4) as epi,
            tc.tile_pool(name="ps_s", bufs=2, space="PSUM") as ps_s,
            tc.tile_pool(name="ps_sm", bufs=4, space="PSUM") as ps_sm,
        ):
            # ---- constants -------------------------------------------------
            wq_sb = const.tile([P, KSUB, HPC * DOUT], BF)
            nc.sync.dma_start(wq_sb[:], wq_d.rearrange("(o p) m -> p o m", p=P))
            wk_sb = const.tile([P, KSUB, HPC * DOUT], BF)
            nc.sync.dma_start(wk_sb[:], wk_d.rearrange("(o p) m -> p o m", p=P))
            wv_sb = const.tile([P, KSUB, HPC * DOUT], BF)
            nc.sync.dma_start(wv_sb[:], wv_d.rearrange("(o p) m -> p o m", p=P))
            wp_sb = const.tile([P, NPAIR, DIN], BF)
            nc.sync.dma_start(wp_sb[:], wp_d.rearrange("(o p) n -> p o n", p=P))
            bqp_sb = const.tile([P, NPAIR], F32)
            nc.sync.dma_start(bqp_sb[:], bqp_d[:])
            bkp_sb = const.tile([P, NPAIR], F32)
            nc.sync.dma_start(bkp_sb[:], bkp_d[:])
            bvr_sb = const.tile([P, HPC * DOUT], F32)
            nc.sync.dma_start(bvr_sb[:], bvr_d[:])
            # full-height ones so any base-partition row is available as a
            # K=1 outer-product lhsT (matmul wants lhsT/rhs base aligned)
            ones_sb = const.tile([P, DOUT], F32)
            nc.vector.memset(ones_sb[:], 1.0)

            # ---- resident transposed activations ---------------------------
            # chunked per DIN-subtile so the first projection matmuls can
            # start as soon as the first 512KB lands
            xq_sb = xin.tile([P, KSUB, SEQ], BF)
            xk_sb = xin.tile([P, KSUB, SEQ], BF)
            xv_sb = xin.tile([P, KSUB, SEQ], BF)
            for o in range(KSUB):
                for x_sb, x_d in ((xq_sb, xq_d), (xk_sb, xk_d), (xv_sb, xv_d)):
                    nc.sync.dma_start(
                        x_sb[:, o, :],
                        x_d.rearrange("(o p) s -> p o s", p=P)[:, o, :],
                    )

            # ---- persistent intermediates ----------------------------------
            qhT = proj.tile([P, NPAIR, SEQ], BF)  # [2-head dout, pair, seq]
            khT = proj.tile([P, NPAIR, SEQ], BF)
            vsb = proj.tile([P, NKT, HPC * VW], BF)  # v + ones col per head
            ynT = proj.tile([P, NPAIR, SEQ], BF)  # normalized y.T

            vsb4 = vsb.rearrange("p k (h c) -> p k h c", c=VW)
            for h in range(HPC):
                nc.vector.memset(vsb4[:, :, h, DOUT : DOUT + 1], 1.0)

            # ---- phase A: projections --------------------------------------
            # q/k head-pair-wise: psum[2*64 dout, 512 seq]
            for pair in range(NPAIR):
                for w_sb, x_sb, b_sb, o_sb in (
                    (wq_sb, xq_sb, bqp_sb, qhT),
                    (wk_sb, xk_sb, bkp_sb, khT),
                ):
                    for qt in range(SEQ // 512):
                        pps = ps_sm.tile([P, 512], F32, tag="sm", name="pps")
                        for o in range(KSUB):
                            nc.tensor.matmul(
                                pps[:],
                                w_sb[:, o, pair * P : (pair + 1) * P],
                                x_sb[:, o, qt * 512 : (qt + 1) * 512],
                                start=(o == 0),
                                stop=(o == KSUB - 1),
                            )
                        nc.vector.tensor_tensor(
                            o_sb[:, pair, qt * 512 : (qt + 1) * 512],
                            pps[:],
                            b_sb[:, pair : pair + 1].to_broadcast((P, 512)),
                            ADD,
                        )
            # v natural layout: psum[128 seq, 256 dout]
            for st in range(NKT):
                vps = ps_sm.tile([P, 512], F32, tag="sm", name="vps")
                for o in range(KSUB):
                    nc.tensor.matmul(
                        vps[:, : HPC * DOUT],
                        xv_sb[:, o, st * P : (st + 1) * P],
                        wv_sb[:, o, :],
                        start=(o == 0),
                        stop=(o == KSUB - 1),
                    )
                nc.vector.tensor_tensor(
                    vsb4[:, st, :, 0:DOUT],
                    vps[:, : HPC * DOUT].rearrange("p (h c) -> p h c", c=DOUT),
                    bvr_sb.rearrange("p (h c) -> p h c", c=DOUT),
                    ADD,
                )

            # ---- phase B: attention ----------------------------------------
            # Normalization epilogues are deferred and flushed mid-way through
            # the NEXT block's kt loop, so their PE outer-product never sits
            # between dense matmul runs waiting on DVE (which would idle the
            # PE long enough to re-throttle the HAM clock gate).
            def flush_epilogue(pending):
                for pv_sb, h2_, pair_, q0 in pending:
                    # denominator broadcast across the 64 dout partitions via
                    # a K=1 outer-product, reciprocal on the broadcast tile
                    # (64 active lanes, not 1), then normalize.
                    ops = ps_s.tile([DOUT, 512], F32, tag="s", name="ops")
                    nc.tensor.matmul(
                        ops[:],
                        ones_sb[DOUT : DOUT + 1, :],
                        pv_sb[DOUT : DOUT + 1, :],
                        start=True,
                        stop=True,
                    )
                    rdb = epi.tile([DOUT, 512], F32, tag="rdb", name="rdb")
                    nc.vector.reciprocal(rdb[:], ops[:])
                    nc.vector.tensor_mul(
                        ynT[h2_ * DOUT : (h2_ + 1) * DOUT, pair_, q0 : q0 + 512],
                        pv_sb[:DOUT, :],
                        rdb[:],
                    )

            pending = []
            for pair in range(NPAIR):
                for qh in range(2):  # halves of the seq_q axis (1024 each)
                    pvs = []
                    for i in range(4):
                        pv = ps_sm.tile([P, 512], F32, tag="sm", name=f"pv{i}")
                        pvs.append(pv)
                    for kt in range(NKT):
                        if kt == 6 and pending:
                            flush_epilogue(pending)
                            pending = []
                        mt = maskp.tile([P, 1024], BF, tag="mt", name="mt")
                        nc.sync.dma_start(
                            mt[:],
                            mk_d[kt * P : (kt + 1) * P, qh * 1024 : (qh + 1) * 1024],
                        )
                        for h2 in range(2):
                            head = pair * 2 + h2
                            hs = slice(h2 * DOUT, (h2 + 1) * DOUT)
                            sps = ps_s.tile([P, 1024], F32, tag="s", name="sps")
                            for qi in range(2):
                                q0 = (qh * 2 + qi) * 512
                                nc.tensor.matmul(
                                    sps[:, qi * 512 : (qi + 1) * 512],
                                    khT[hs, pair, kt * P : (kt + 1) * P],
                                    qhT[hs, pair, q0 : q0 + 512],
                                    start=True,
                                    stop=True,
                                )
                            pt = ptp.tile([P, 1024], BF, tag="pt", name="pt")
                            nc.scalar.activation(pt[:], sps[:], EXP, scale=0.125)
                            ptm = ptp.tile([P, 1024], BF, tag="ptm", name="ptm")
                            nc.vector.tensor_mul(ptm[:], pt[:], mt[:])
                            for qi in range(2):
                                nc.tensor.matmul(
                                    pvs[h2 * 2 + qi][: DOUT + 1, :],
                                    vsb4[:, kt, head, :],
                                    ptm[:, qi * 512 : (qi + 1) * 512],
                                    start=(kt == 0),
                                    stop=(kt == NKT - 1),
                                )
                    # drain PV psums to SBUF promptly to free the psum slots;
                    # normalization itself is deferred (see flush above)
                    for h2 in range(2):
                        for qi in range(2):
                            pv_sb = epi.tile(
                                [DOUT + 1, 512], F32, tag="pvsb", name="pvsb", bufs=8
                            )
                            nc.vector.tensor_copy(
                                pv_sb[:], pvs[h2 * 2 + qi][: DOUT + 1, :]
                            )
                            pending.append((pv_sb, h2, pair, (qh * 2 + qi) * 512))
            flush_epilogue(pending)
            pending = []

            # ---- phase C: output projection (partial over 4 heads) ---------
            for st in range(NKT):
                for ntile in range(DIN // 512):
                    cps = ps_sm.tile([P, 512], F32, tag="sm", name="cps")
                    for pair in range(NPAIR):
                        nc.tensor.matmul(
                            cps[:],
                            ynT[:, pair, st * P : (st + 1) * P],
                            wp_sb[:, pair, ntile * 512 : (ntile + 1) * 512],
                            start=(pair == 0),
                            stop=(pair == NPAIR - 1),
                        )
                    ost = epi.tile([P, 512], F32, tag="ost", name="ost", bufs=3)
                    nc.scalar.copy(ost[:], cps[:])
                    nc.sync.dma_start(
                        out_d[st * P : (st + 1) * P, ntile * 512 : (ntile + 1) * 512],
                        ost[:],
                    )

    nc.finalize()
    return nc


def make_in_maps(query, key, value, mask, Wq, bq, Wk, bk, Wv, bv, Wp, bp):
    """Shard + pre-layout the full inputs into 8 per-core input dicts."""
    in_maps = []
    for c in range(8):
        n = c // 4
        h0 = HPC * (c % 4)
        hs = slice(h0, h0 + HPC)

        def t_bf(x):  # [SEQ, DIN] -> contiguous [DIN, SEQ] bf16
            return np.ascontiguousarray(x.T).astype(BF_NP)

        # (H', DIN, DOUT) -> (DIN, H'*DOUT), head-major columns
        def w_bf(W):
            return np.ascontiguousarray(
                W[hs].transpose(1, 0, 2).reshape(DIN, HPC * DOUT)
            ).astype(BF_NP)

        # per-pair per-partition bias: [128, 2], col p = concat of heads (2p, 2p+1)
        def b_pair(b):
            return np.ascontiguousarray(b[hs].reshape(NPAIR, P).T).astype(np.float32)

        in_maps.append(
            {
                "xqT": t_bf(query[n]),
                "xkT": t_bf(key[n]),
                "xvT": t_bf(value[n]),
                "maskT": np.ascontiguousarray((~mask[n]).T).astype(BF_NP),
                "wq": w_bf(Wq),
                "wk": w_bf(Wk),
                "wv": w_bf(Wv),
                "wp": np.ascontiguousarray(
                    Wp[h0 * DOUT : (h0 + HPC) * DOUT, :]
                ).astype(BF_NP),
                "bqp": b_pair(bq),
                "bkp": b_pair(bk),
                "bvr": np.ascontiguousarray(
                    np.tile(bv[hs].reshape(1, HPC * DOUT), (P, 1))
                ).astype(np.float32),
            }
        )
    return in_maps


def kernel(**inputs):
    global _NC_CACHE
    from concourse.bass_utils import run_bass_kernel_spmd

    if _NC_CACHE is None:
        _NC_CACHE = build_bass()
    nc = _NC_CACHE

    in_maps = make_in_maps(**inputs)
    res = run_bass_kernel_spmd(nc, in_maps, core_ids=list(range(8))).results
    parts = [res[c]["out"].astype(np.float32) for c in range(8)]
    bp = inputs["bp"]
    out = np.stack(
        [
            parts[0] + parts[1] + parts[2] + parts[3] + bp[None, :],
            parts[4] + parts[5] + parts[6] + parts[7] + bp[None, :],
        ]
    )
    return out.astype(np.float32)


# revision 34
# speedup vs baseline: 1.0306x; 1.0056x over previous
"""Multi-head attention (N=2, K=2048, DIN=1024, H=16, DOUT=64) on 8 TRN2 NeuronCores.

Sharding: data-parallel over batch N (cores 0-3 -> n=0, cores 4-7 -> n=1),
tensor-parallel over heads (4 heads per core). Each core computes its 4 heads'
attention plus the partial output projection over its head-feature slice of Wp;
the host sums the 4 partials per batch element and adds the output bias.

Per-core kernel structure (all matmuls bf16, fp32 PSUM accumulation):
  - host pre-transposes/casts activations to bf16 [DIN, SEQ] so projection
    matmuls contract over DIN on partitions with natural contiguous DMA.
  - q/k projected head-pair-wise to [dout(2 heads on partitions), seq];
    v projected to the natural [seq, 4*64] layout.
  - scores computed transposed (S.T = k_h @ q_h.T: seq_k on partitions), so
    softmax probabilities are already in the layout the PV matmul needs.
  - no max-subtraction: scores are bounded (~|S/8| < 10), exp is safe in fp32.
  - mask applied as a 0/1 multiply AFTER exp (identical to -1e9 pre-masking).
  - the two heads' PV matmuls col-pack into one PSUM bank (concurrent column
    groups); softmax denominators accumulate via col-packed M=1 ones-matmuls
    in a separate shared bank. This keeps the TensorEngine the near-100%-busy
    pacer of the attention phase so the HAM clock gate stays at full clock.
  - normalization: reciprocal of the denominators, one K=2 outer-product
    broadcast per 512-col block (kmask selects which head-half each dout
    partition group gets), then a single [128,512] multiply for both heads.
  - epilogues are deferred into the middle of the NEXT block's matmul stream
    so the PE never sits idle long enough to re-throttle.
"""

import numpy as np
import ml_dtypes

import concourse.bass as bass
import concourse.mybir as mybir
from concourse import bacc
from concourse.tile import TileContext

P = 128
SEQ = 2048
DIN = 1024
DOUT = 64
H = 16
N = 2
HPC = 4  # heads per core
NPAIR = 2  # head pairs per core
KSUB = DIN // P  # 8 contraction subtiles for projections
NKT = SEQ // P  # 16 seq_k tiles of 128
BF = mybir.dt.bfloat16
F32 = mybir.dt.float32
BF_NP = ml_dtypes.bfloat16

_NC_CACHE = None


def build_bass():
    nc = bacc.Bacc()

    xq_d = nc.declare_dram_parameter("xqT", [DIN, SEQ], BF, isOutput=False)
    xk_d = nc.declare_dram_parameter("xkT", [DIN, SEQ], BF, isOutput=False)
    xv_d = nc.declare_dram_parameter("xvT", [DIN, SEQ], BF, isOutput=False)
    mk_d = nc.declare_dram_parameter("maskT", [SEQ, SEQ], BF, isOutput=False)
    wq_d = nc.declare_dram_parameter("wq", [DIN, HPC * DOUT], BF, isOutput=False)
    wk_d = nc.declare_dram_parameter("wk", [DIN, HPC * DOUT], BF, isOutput=False)
    wv_d = nc.declare_dram_parameter("wv", [DIN, HPC * DOUT], BF, isOutput=False)
    wp_d = nc.declare_dram_parameter("wp", [HPC * DOUT, DIN], BF, isOutput=False)
    bqp_d = nc.declare_dram_parameter("bqp", [P, NPAIR], F32, isOutput=False)
    bkp_d = nc.declare_dram_parameter("bkp", [P, NPAIR], F32, isOutput=False)
    bvr_d = nc.declare_dram_parameter("bvr", [P, HPC * DOUT], F32, isOutput=False)
    out_d = nc.declare_dram_parameter("out", [SEQ, DIN], F32, isOutput=True)

    ADD = mybir.AluOpType.add
    EXP = mybir.ActivationFunctionType.Exp

    with TileContext(nc) as tc:
        with (
            tc.tile_pool(name="const", bufs=1) as const,
            tc.tile_pool(name="xin", bufs=1) as xin,
            tc.tile_pool(name="proj", bufs=1) as proj,
            tc.tile_pool(name="maskp", bufs=3) as maskp,
            tc.tile_pool(name="ptp", bufs=2) as ptp,
            tc.tile_pool(name="epi", bufs=2) as epi,
            tc.tile_pool(name="ps_s", bufs=2, space="PSUM") as ps_s,
            tc.tile_pool(name="ps_pv", bufs=2, space="PSUM") as ps_pv,
            tc.tile_pool(name="ps_dn", bufs=2, space="PSUM") as ps_dn,
        ):
            # ---- constants -------------------------------------------------
            wq_sb = const.tile([P, KSUB, HPC * DOUT], BF)
            nc.sync.dma_start(wq_sb[:], wq_d.rearrange("(o p) m -> p o m", p=P))
            wk_sb = const.tile([P, KSUB, HPC * DOUT], BF)
            nc.sync.dma_start(wk_sb[:], wk_d.rearrange("(o p) m -> p o m", p=P))
            wv_sb = const.tile([P, KSUB, HPC * DOUT], BF)
            nc.sync.dma_start(wv_sb[:], wv_d.rearrange("(o p) m -> p o m", p=P))
            wp_sb = const.tile([P, NPAIR, DIN], BF)
            nc.sync.dma_start(wp_sb[:], wp_d.rearrange("(o p) n -> p o n", p=P))
            bqp_sb = const.tile([P, NPAIR], F32)
            nc.sync.dma_start(bqp_sb[:], bqp_d[:])
            bkp_sb = const.tile([P, NPAIR], F32)
            nc.sync.dma_start(bkp_sb[:], bkp_d[:])
            bvr_sb = const.tile([P, HPC * DOUT], F32)
            nc.sync.dma_start(bvr_sb[:], bvr_d[:])
            ones_sb = const.tile([P, 1], BF)
            nc.vector.memset(ones_sb[:], 1.0)
            # full-height fp32 ones: any 32-aligned row serves as a K=1
            # outer-product lhsT whose base partition matches the rhs row
            ones64_sb = const.tile([P, DOUT], F32)
            nc.vector.memset(ones64_sb[:], 1.0)

            # ---- resident transposed activations ---------------------------
            # chunked per DIN-subtile so the first projection matmuls can
            # start as soon as the first 512KB lands
            xq_sb = xin.tile([P, KSUB, SEQ], BF)
            xk_sb = xin.tile([P, KSUB, SEQ], BF)
            xv_sb = xin.tile([P, KSUB, SEQ], BF)
            for o in range(KSUB):
                for x_sb, x_d in ((xq_sb, xq_d), (xk_sb, xk_d), (xv_sb, xv_d)):
                    nc.sync.dma_start(
                        x_sb[:, o, :],
                        x_d.rearrange("(o p) s -> p o s", p=P)[:, o, :],
                    )

            # ---- persistent intermediates ----------------------------------
            qhT = proj.tile([P, NPAIR, SEQ], BF)  # [2-head dout, pair, seq]
            khT = proj.tile([P, NPAIR, SEQ], BF)
            vsb = proj.tile([P, NKT, HPC * DOUT], BF)  # v natural [seq, dout]
            ynT = proj.tile([P, NPAIR, SEQ], BF)  # normalized y.T

            vsb4 = vsb.rearrange("p k (h c) -> p k h c", c=DOUT)

            # ---- phase A: projections --------------------------------------
            # q/k head-pair-wise: psum[2*64 dout, 512 seq]
            for pair in range(NPAIR):
                for w_sb, x_sb, b_sb, o_sb in (
                    (wq_sb, xq_sb, bqp_sb, qhT),
                    (wk_sb, xk_sb, bkp_sb, khT),
                ):
                    for qt in range(SEQ // 512):
                        pps = ps_pv.tile([P, 512], F32, tag="pv", name="pps")
                        for o in range(KSUB):
                            nc.tensor.matmul(
                                pps[:],
                                w_sb[:, o, pair * P : (pair + 1) * P],
                                x_sb[:, o, qt * 512 : (qt + 1) * 512],
                                start=(o == 0),
                                stop=(o == KSUB - 1),
                            )
                        nc.vector.tensor_tensor(
                            o_sb[:, pair, qt * 512 : (qt + 1) * 512],
                            pps[:],
                            b_sb[:, pair : pair + 1].to_broadcast((P, 512)),
                            ADD,
                        )
            # v natural layout: psum[128 seq, 256 dout]
            for st in range(NKT):
                vps = ps_pv.tile([P, 512], F32, tag="pv", name="vps")
                for o in range(KSUB):
                    nc.tensor.matmul(
                        vps[:, : HPC * DOUT],
                        xv_sb[:, o, st * P : (st + 1) * P],
                        wv_sb[:, o, :],
                        start=(o == 0),
                        stop=(o == KSUB - 1),
                    )
                nc.vector.tensor_tensor(
                    vsb4[:, st, :, :],
                    vps[:, : HPC * DOUT].rearrange("p (h c) -> p h c", c=DOUT),
                    bvr_sb.rearrange("p (h c) -> p h c", c=DOUT),
                    ADD,
                )

            # ---- phase B: attention ----------------------------------------
            # Epilogues are deferred and flushed mid-way through the NEXT
            # block's kt loop so their PE outer-products never pace-block the
            # dense matmul stream.
            def flush_epilogue(pending):
                for dn, pv_sbs, pair_, qh_ in pending:
                    for qi in range(2):
                        q0 = (qh_ * 2 + qi) * 512
                        for h2 in range(2):
                            # 1/den straight from the psum row into a base-0
                            # SBUF row, broadcast across the head's 64 dout
                            # rows via a K=1 outer-product, then normalize
                            row = 32 * qi + 64 * h2
                            rcp = epi.tile([1, 512], F32, tag="rcp", name="rcp", bufs=4)
                            nc.vector.reciprocal(rcp[:], dn[row : row + 1, :])
                            rdb = ps_s.tile([DOUT, 512], F32, tag="s", name="rdb")
                            nc.tensor.matmul(
                                rdb[:],
                                ones64_sb[0:1, :],
                                rcp[:],
                                start=True,
                                stop=True,
                            )
                            nc.vector.tensor_mul(
                                ynT[
                                    h2 * DOUT : (h2 + 1) * DOUT,
                                    pair_,
                                    q0 : q0 + 512,
                                ],
                                pv_sbs[qi][h2 * DOUT : (h2 + 1) * DOUT, :],
                                rdb[:],
                            )

            pending = []
            for pair in range(NPAIR):
                for qh in range(2):  # halves of the seq_q axis (1024 each)
                    # accumulators are DVE-zeroed and every matmul uses
                    # start=False: correct regardless of whether the HW
                    # has_written clear is bank-wide or per-region
                    pvs = []
                    for i in range(2):
                        pv = ps_pv.tile([P, 512], F32, tag="pv", name=f"pv{i}")
                        nc.vector.memset(pv[:], 0.0)
                        pvs.append(pv)
                    dn = ps_dn.tile([P, 512], F32, tag="dn", name="dn")
                    nc.vector.memset(dn[:], 0.0)
                    for kt in range(NKT):
                        if kt == 6 and pending:
                            flush_epilogue(pending)
                            pending = []
                        mt = maskp.tile([P, 1024], BF, tag="mt", name="mt")
                        nc.sync.dma_start(
                            mt[:],
                            mk_d[kt * P : (kt + 1) * P, qh * 1024 : (qh + 1) * 1024],
                        )
                        ptms = []
                        for h2 in range(2):
                            hs = slice(h2 * DOUT, (h2 + 1) * DOUT)
                            sps = ps_s.tile([P, 1024], F32, tag="s", name="sps")
                            for qi in range(2):
                                q0 = (qh * 2 + qi) * 512
                                nc.tensor.matmul(
                                    sps[:, qi * 512 : (qi + 1) * 512],
                                    khT[hs, pair, kt * P : (kt + 1) * P],
                                    qhT[hs, pair, q0 : q0 + 512],
                                    start=True,
                                    stop=True,
                                )
                            pt = ptp.tile([P, 1024], BF, tag=f"pt{h2}", name="pt")
                            nc.scalar.activation(pt[:], sps[:], EXP, scale=0.125)
                            ptm = ptp.tile([P, 1024], BF, tag=f"ptm{h2}", name="ptm")
                            nc.vector.tensor_mul(ptm[:], pt[:], mt[:])
                            ptms.append(ptm)
                        for qi in range(2):
                            for h2 in range(2):
                                # col-packed: head h2 -> psum partitions
                                # h2*64..h2*64+63; the two matmuls run in
                                # distinct column groups concurrently
                                # the has_written "start" clear is per
                                # partition-region, so each column group
                                # starts its own accumulation run
                                nc.tensor.matmul(
                                    pvs[qi][h2 * DOUT : (h2 + 1) * DOUT, :],
                                    vsb4[:, kt, pair * 2 + h2, :],
                                    ptms[h2][:, qi * 512 : (qi + 1) * 512],
                                    start=False,
                                    stop=(kt == NKT - 1),
                                    tile_position=(0, h2 * DOUT),
                                    skip_group_check=True,
                                )
                        for qi in range(2):
                            for h2 in range(2):
                                row = 32 * qi + 64 * h2
                                nc.tensor.matmul(
                                    dn[row : row + 1, :],
                                    ones_sb[:],
                                    ptms[h2][:, qi * 512 : (qi + 1) * 512],
                                    start=False,
                                    stop=(kt == NKT - 1),
                                    tile_position=(0, row),
                                    skip_group_check=True,
                                )
                    # drain accumulators to SBUF promptly to free psum slots;
                    # normalization itself is deferred (see flush above)
                    pv_sbs = []
                    for qi in range(2):
                        pv_sb = epi.tile([P, 512], F32, tag="pvsb", name="pv_sb", bufs=4)
                        nc.vector.tensor_copy(pv_sb[:], pvs[qi][:])
                        pv_sbs.append(pv_sb)
                    pending.append((dn, pv_sbs, pair, qh))
            flush_epilogue(pending)
            pending = []

            # ---- phase C: output projection (partial over 4 heads) ---------
            for st in range(NKT):
                for ntile in range(DIN // 512):
                    cps = ps_pv.tile([P, 512], F32, tag="pv", name="cps")
                    for pair in range(NPAIR):
                        nc.tensor.matmul(
                            cps[:],
                            ynT[:, pair, st * P : (st + 1) * P],
                            wp_sb[:, pair, ntile * 512 : (ntile + 1) * 512],
                            start=(pair == 0),
                            stop=(pair == NPAIR - 1),
                        )
                    ost = epi.tile([P, 512], F32, tag="ost", name="ost", bufs=3)
                    nc.scalar.copy(ost[:], cps[:])
                    nc.sync.dma_start(
                        out_d[st * P : (st + 1) * P, ntile * 512 : (ntile + 1) * 512],
                        ost[:],
                    )

    nc.finalize()
    return nc


def make_in_maps(query, key, value, mask, Wq, bq, Wk, bk, Wv, bv, Wp, bp):
    """Shard + pre-layout the full inputs into 8 per-core input dicts."""
    in_maps = []
    for c in range(8):
        n = c // 4
        h0 = HPC * (c % 4)
        hs = slice(h0, h0 + HPC)

        def t_bf(x):  # [SEQ, DIN] -> contiguous [DIN, SEQ] bf16
            return np.ascontiguousarray(x.T).astype(BF_NP)

        # (H', DIN, DOUT) -> (DIN, H'*DOUT), head-major columns
        def w_bf(W):
            return np.ascontiguousarray(
                W[hs].transpose(1, 0, 2).reshape(DIN, HPC * DOUT)
            ).astype(BF_NP)

        # per-pair per-partition bias: [128, 2], col p = concat of heads (2p, 2p+1)
        def b_pair(b):
            return np.ascontiguousarray(b[hs].reshape(NPAIR, P).T).astype(np.float32)

        in_maps.append(
            {
                "xqT": t_bf(query[n]),
                "xkT": t_bf(key[n]),
                "xvT": t_bf(value[n]),
                "maskT": np.ascontiguousarray((~mask[n]).T).astype(BF_NP),
                "wq": w_bf(Wq),
                "wk": w_bf(Wk),
                "wv": w_bf(Wv),
                "wp": np.ascontiguousarray(
                    Wp[h0 * DOUT : (h0 + HPC) * DOUT, :]
                ).astype(BF_NP),
                "bqp": b_pair(bq),
                "bkp": b_pair(bk),
                "bvr": np.ascontiguousarray(
                    np.tile(bv[hs].reshape(1, HPC * DOUT), (P, 1))
                ).astype(np.float32),
            }
        )
    return in_maps


def kernel(**inputs):
    global _NC_CACHE
    from concourse.bass_utils import run_bass_kernel_spmd

    if _NC_CACHE is None:
        _NC_CACHE = build_bass()
    nc = _NC_CACHE

    in_maps = make_in_maps(**inputs)
    res = run_bass_kernel_spmd(nc, in_maps, core_ids=list(range(8))).results
    parts = [res[c]["out"].astype(np.float32) for c in range(8)]
    bp = inputs["bp"]
    out = np.stack(
        [
            parts[0] + parts[1] + parts[2] + parts[3] + bp[None, :],
            parts[4] + parts[5] + parts[6] + parts[7] + bp[None, :],
        ]
    )
    return out.astype(np.float32)


# revision 35
# speedup vs baseline: 1.1928x; 1.1574x over previous
"""Multi-head attention (N=2, K=2048, DIN=1024, H=16, DOUT=64) on 8 TRN2 NeuronCores.

Sharding: data-parallel over batch N (cores 0-3 -> n=0, cores 4-7 -> n=1),
tensor-parallel over heads (4 heads per core). Each core computes its 4 heads'
attention plus the partial output projection over its head-feature slice of Wp;
the host sums the 4 partials per batch element and adds the output bias.

Per-core kernel structure (all matmuls bf16, fp32 PSUM accumulation):
  - host pre-transposes/casts activations to bf16 [DIN, SEQ] so projection
    matmuls contract over DIN on partitions with natural contiguous DMA.
  - q/k projected head-pair-wise to [dout(2 heads on partitions), seq];
    v projected to the natural [seq, 4*64] layout.
  - scores computed transposed (S.T = k_h @ q_h.T: seq_k on partitions), so
    softmax probabilities are already in the layout the PV matmul needs.
  - no max-subtraction: scores are bounded (~|S/8| < 10), exp is safe in fp32.
  - mask applied as a 0/1 multiply AFTER exp (identical to -1e9 pre-masking).
  - the two heads' PV matmuls col-pack into one PSUM bank (concurrent column
    groups); softmax denominators accumulate via col-packed M=1 ones-matmuls
    in a separate shared bank. This keeps the TensorEngine the near-100%-busy
    pacer of the attention phase so the HAM clock gate stays at full clock.
  - normalization: reciprocal of the denominators, one K=2 outer-product
    broadcast per 512-col block (kmask selects which head-half each dout
    partition group gets), then a single [128,512] multiply for both heads.
  - epilogues are deferred into the middle of the NEXT block's matmul stream
    so the PE never sits idle long enough to re-throttle.
"""

import numpy as np
import ml_dtypes

import concourse.bass as bass
import concourse.mybir as mybir
from concourse import bacc
from concourse.tile import TileContext

P = 128
SEQ = 2048
DIN = 1024
DOUT = 64
H = 16
N = 2
HPC = 4  # heads per core
NPAIR = 2  # head pairs per core
KSUB = DIN // P  # 8 contraction subtiles for projections
NKT = SEQ // P  # 16 seq_k tiles of 128
BF = mybir.dt.bfloat16
F32 = mybir.dt.float32
BF_NP = ml_dtypes.bfloat16

_NC_CACHE = None


def build_bass():
    nc = bacc.Bacc()

    xq_d = nc.declare_dram_parameter("xqT", [DIN, SEQ], BF, isOutput=False)
    xk_d = nc.declare_dram_parameter("xkT", [DIN, SEQ], BF, isOutput=False)
    xv_d = nc.declare_dram_parameter("xvT", [DIN, SEQ], BF, isOutput=False)
    mk_d = nc.declare_dram_parameter("maskT", [SEQ, SEQ], BF, isOutput=False)
    wq_d = nc.declare_dram_parameter("wq", [DIN, HPC * DOUT], BF, isOutput=False)
    wk_d = nc.declare_dram_parameter("wk", [DIN, HPC * DOUT], BF, isOutput=False)
    wv_d = nc.declare_dram_parameter("wv", [DIN, HPC * DOUT], BF, isOutput=False)
    wp_d = nc.declare_dram_parameter("wp", [HPC * DOUT, DIN], BF, isOutput=False)
    bqp_d = nc.declare_dram_parameter("bqp", [P, NPAIR], F32, isOutput=False)
    bkp_d = nc.declare_dram_parameter("bkp", [P, NPAIR], F32, isOutput=False)
    bvr_d = nc.declare_dram_parameter("bvr", [P, HPC * DOUT], F32, isOutput=False)
    out_d = nc.declare_dram_parameter("out", [SEQ, DIN], F32, isOutput=True)

    ADD = mybir.AluOpType.add
    EXP = mybir.ActivationFunctionType.Exp

    with TileContext(nc) as tc:
        with (
            tc.tile_pool(name="const", bufs=1) as const,
            tc.tile_pool(name="xin", bufs=1) as xin,
            tc.tile_pool(name="proj", bufs=1) as proj,
            tc.tile_pool(name="maskp", bufs=3) as maskp,
            tc.tile_pool(name="ptp", bufs=2) as ptp,
            tc.tile_pool(name="epi", bufs=2) as epi,
            tc.tile_pool(name="ps_s", bufs=2, space="PSUM") as ps_s,
            tc.tile_pool(name="ps_pv", bufs=2, space="PSUM") as ps_pv,
            tc.tile_pool(name="ps_dn", bufs=2, space="PSUM") as ps_dn,
        ):
            # ---- constants -------------------------------------------------
            wq_sb = const.tile([P, KSUB, HPC * DOUT], BF)
            nc.sync.dma_start(wq_sb[:], wq_d.rearrange("(o p) m -> p o m", p=P))
            wk_sb = const.tile([P, KSUB, HPC * DOUT], BF)
            nc.sync.dma_start(wk_sb[:], wk_d.rearrange("(o p) m -> p o m", p=P))
            wv_sb = const.tile([P, KSUB, HPC * DOUT], BF)
            nc.sync.dma_start(wv_sb[:], wv_d.rearrange("(o p) m -> p o m", p=P))
            wp_sb = const.tile([P, NPAIR, DIN], BF)
            nc.sync.dma_start(wp_sb[:], wp_d.rearrange("(o p) n -> p o n", p=P))
            bqp_sb = const.tile([P, NPAIR], F32)
            nc.sync.dma_start(bqp_sb[:], bqp_d[:])
            bkp_sb = const.tile([P, NPAIR], F32)
            nc.sync.dma_start(bkp_sb[:], bkp_d[:])
            bvr_sb = const.tile([P, HPC * DOUT], F32)
            nc.sync.dma_start(bvr_sb[:], bvr_d[:])
            ones_sb = const.tile([P, 1], BF)
            nc.vector.memset(ones_sb[:], 1.0)
            # full-height fp32 ones: any 32-aligned row serves as a K=1
            # outer-product lhsT whose base partition matches the rhs row
            ones64_sb = const.tile([P, DOUT], F32)
            nc.vector.memset(ones64_sb[:], 1.0)

            # ---- resident transposed activations ---------------------------
            # chunked per DIN-subtile so the first projection matmuls can
            # start as soon as the first 512KB lands
            xq_sb = xin.tile([P, KSUB, SEQ], BF)
            xk_sb = xin.tile([P, KSUB, SEQ], BF)
            xv_sb = xin.tile([P, KSUB, SEQ], BF)
            for o in range(KSUB):
                for x_sb, x_d in ((xq_sb, xq_d), (xk_sb, xk_d), (xv_sb, xv_d)):
                    nc.sync.dma_start(
                        x_sb[:, o, :],
                        x_d.rearrange("(o p) s -> p o s", p=P)[:, o, :],
                    )

            # ---- persistent intermediates ----------------------------------
            qhT = proj.tile([P, NPAIR, SEQ], BF)  # [2-head dout, pair, seq]
            khT = proj.tile([P, NPAIR, SEQ], BF)
            vsb = proj.tile([P, NKT, HPC * DOUT], BF)  # v natural [seq, dout]
            ynT = proj.tile([P, NPAIR, SEQ], BF)  # normalized y.T

            vsb4 = vsb.rearrange("p k (h c) -> p k h c", c=DOUT)

            # ---- phase A: projections --------------------------------------
            # q/k head-pair-wise: psum[2*64 dout, 512 seq]
            for pair in range(NPAIR):
                for w_sb, x_sb, b_sb, o_sb in (
                    (wq_sb, xq_sb, bqp_sb, qhT),
                    (wk_sb, xk_sb, bkp_sb, khT),
                ):
                    for qt in range(SEQ // 512):
                        pps = ps_pv.tile([P, 512], F32, tag="pv", name="pps")
                        for o in range(KSUB):
                            nc.tensor.matmul(
                                pps[:],
                                w_sb[:, o, pair * P : (pair + 1) * P],
                                x_sb[:, o, qt * 512 : (qt + 1) * 512],
                                start=(o == 0),
                                stop=(o == KSUB - 1),
                            )
                        nc.vector.tensor_tensor(
                            o_sb[:, pair, qt * 512 : (qt + 1) * 512],
                            pps[:],
                            b_sb[:, pair : pair + 1].to_broadcast((P, 512)),
                            ADD,
                        )
            # v natural layout: psum[128 seq, 256 dout]
            for st in range(NKT):
                vps = ps_pv.tile([P, 512], F32, tag="pv", name="vps")
                for o in range(KSUB):
                    nc.tensor.matmul(
                        vps[:, : HPC * DOUT],
                        xv_sb[:, o, st * P : (st + 1) * P],
                        wv_sb[:, o, :],
                        start=(o == 0),
                        stop=(o == KSUB - 1),
                    )
                nc.vector.tensor_tensor(
                    vsb4[:, st, :, :],
                    vps[:, : HPC * DOUT].rearrange("p (h c) -> p h c", c=DOUT),
                    bvr_sb.rearrange("p (h c) -> p h c", c=DOUT),
                    ADD,
                )

            # ---- phase B: attention ----------------------------------------
            # Epilogues are deferred and flushed mid-way through the NEXT
            # block's kt loop so their PE outer-products never pace-block the
            # dense matmul stream.
            def flush_epilogue(pending):
                for dn, pv_sbs, pair_, qh_ in pending:
                    for qi in range(2):
                        q0 = (qh_ * 2 + qi) * 512
                        for h2 in range(2):
                            # 1/den straight from the psum row into a base-0
                            # SBUF row, broadcast across the head's 64 dout
                            # rows via a K=1 outer-product, then normalize
                            row = 32 * qi + 64 * h2
                            den = epi.tile([1, 512], F32, tag="den", name="den", bufs=4)
                            nc.vector.tensor_copy(den[:], dn[row : row + 1, :])
                            rcp = epi.tile([1, 512], F32, tag="rcp", name="rcp", bufs=4)
                            nc.vector.reciprocal_approx_fast(rcp[:], den[:])
                            rdb = ps_s.tile([DOUT, 512], F32, tag="s", name="rdb")
                            nc.tensor.matmul(
                                rdb[:],
                                ones64_sb[0:1, :],
                                rcp[:],
                                start=True,
                                stop=True,
                            )
                            nc.vector.tensor_mul(
                                ynT[
                                    h2 * DOUT : (h2 + 1) * DOUT,
                                    pair_,
                                    q0 : q0 + 512,
                                ],
                                pv_sbs[qi][h2 * DOUT : (h2 + 1) * DOUT, :],
                                rdb[:],
                            )

            pending = []
            for pair in range(NPAIR):
                for qh in range(2):  # halves of the seq_q axis (1024 each)
                    # accumulators are DVE-zeroed and every matmul uses
                    # start=False: correct regardless of whether the HW
                    # has_written clear is bank-wide or per-region
                    pvs = []
                    for i in range(2):
                        pv = ps_pv.tile([P, 512], F32, tag="pv", name=f"pv{i}")
                        nc.vector.memset(pv[:], 0.0)
                        pvs.append(pv)
                    dn = ps_dn.tile([P, 512], F32, tag="dn", name="dn")
                    nc.vector.memset(dn[:], 0.0)
                    for kt in range(NKT):
                        if kt == 6 and pending:
                            flush_epilogue(pending)
                            pending = []
                        mt = maskp.tile([P, 1024], BF, tag="mt", name="mt")
                        nc.sync.dma_start(
                            mt[:],
                            mk_d[kt * P : (kt + 1) * P, qh * 1024 : (qh + 1) * 1024],
                        )
                        ptms = []
                        for h2 in range(2):
                            hs = slice(h2 * DOUT, (h2 + 1) * DOUT)
                            sps = ps_s.tile([P, 1024], F32, tag="s", name="sps")
                            for qi in range(2):
                                q0 = (qh * 2 + qi) * 512
                                nc.tensor.matmul(
                                    sps[:, qi * 512 : (qi + 1) * 512],
                                    khT[hs, pair, kt * P : (kt + 1) * P],
                                    qhT[hs, pair, q0 : q0 + 512],
                                    start=True,
                                    stop=True,
                                )
                            pt = ptp.tile([P, 1024], BF, tag=f"pt{h2}", name="pt")
                            nc.scalar.activation(pt[:], sps[:], EXP, scale=0.125)
                            ptm = ptp.tile([P, 1024], BF, tag=f"ptm{h2}", name="ptm")
                            nc.vector.tensor_mul(ptm[:], pt[:], mt[:])
                            ptms.append(ptm)
                        for qi in range(2):
                            for h2 in range(2):
                                # col-packed: head h2 -> psum partitions
                                # h2*64..h2*64+63; the two matmuls run in
                                # distinct column groups concurrently
                                # the has_written "start" clear is per
                                # partition-region, so each column group
                                # starts its own accumulation run
                                nc.tensor.matmul(
                                    pvs[qi][h2 * DOUT : (h2 + 1) * DOUT, :],
                                    vsb4[:, kt, pair * 2 + h2, :],
                                    ptms[h2][:, qi * 512 : (qi + 1) * 512],
                                    start=False,
                                    stop=(kt == NKT - 1),
                                    tile_position=(0, h2 * DOUT),
                                    skip_group_check=True,
                                )
                        for qi in range(2):
                            for h2 in range(2):
                                row = 32 * qi + 64 * h2
                                nc.tensor.matmul(
                                    dn[row : row + 1, :],
                                    ones_sb[:],
                                    ptms[h2][:, qi * 512 : (qi + 1) * 512],
                                    start=False,
                                    stop=(kt == NKT - 1),
                                    tile_position=(0, row),
                                    skip_group_check=True,
                                )
                    # drain accumulators to SBUF promptly to free psum slots;
                    # normalization itself is deferred (see flush above)
                    pv_sbs = []
                    for qi in range(2):
                        pv_sb = epi.tile([P, 512], F32, tag="pvsb", name="pv_sb", bufs=4)
                        nc.vector.tensor_copy(pv_sb[:], pvs[qi][:])
                        pv_sbs.append(pv_sb)
                    pending.append((dn, pv_sbs, pair, qh))
            flush_epilogue(pending)
            pending = []

            # ---- phase C: output projection (partial over 4 heads) ---------
            for st in range(NKT):
                for ntile in range(DIN // 512):
                    cps = ps_pv.tile([P, 512], F32, tag="pv", name="cps")
                    for pair in range(NPAIR):
                        nc.tensor.matmul(
                            cps[:],
                            ynT[:, pair, st * P : (st + 1) * P],
                            wp_sb[:, pair, ntile * 512 : (ntile + 1) * 512],
                            start=(pair == 0),
                            stop=(pair == NPAIR - 1),
                        )
                    ost = epi.tile([P, 512], F32, tag="ost", name="ost", bufs=3)
                    nc.scalar.copy(ost[:], cps[:])
                    nc.sync.dma_start(
                        out_d[st * P : (st + 1) * P, ntile * 512 : (ntile + 1) * 512],
                        ost[:],
                    )

    nc.finalize()
    return nc


def make_in_maps(query, key, value, mask, Wq, bq, Wk, bk, Wv, bv, Wp, bp):
    """Shard + pre-layout the full inputs into 8 per-core input dicts."""
    in_maps = []
    for c in range(8):
        n = c // 4
        h0 = HPC * (c % 4)
        hs = slice(h0, h0 + HPC)

        def t_bf(x):  # [SEQ, DIN] -> contiguous [DIN, SEQ] bf16
            return np.ascontiguousarray(x.T).astype(BF_NP)

        # (H', DIN, DOUT) -> (DIN, H'*DOUT), head-major columns
        def w_bf(W):
            return np.ascontiguousarray(
                W[hs].transpose(1, 0, 2).reshape(DIN, HPC * DOUT)
            ).astype(BF_NP)

        # per-pair per-partition bias: [128, 2], col p = concat of heads (2p, 2p+1)
        def b_pair(b):
            return np.ascontiguousarray(b[hs].reshape(NPAIR, P).T).astype(np.float32)

        in_maps.append(
            {
                "xqT": t_bf(query[n]),
                "xkT": t_bf(key[n]),
                "xvT": t_bf(value[n]),
                "maskT": np.ascontiguousarray((~mask[n]).T).astype(BF_NP),
                "wq": w_bf(Wq),
                "wk": w_bf(Wk),
                "wv": w_bf(Wv),
                "wp": np.ascontiguousarray(
                    Wp[h0 * DOUT : (h0 + HPC) * DOUT, :]
                ).astype(BF_NP),
                "bqp": b_pair(bq),
                "bkp": b_pair(bk),
                "bvr": np.ascontiguousarray(
                    np.tile(bv[hs].reshape(1, HPC * DOUT), (P, 1))
                ).astype(np.float32),
            }
        )
    return in_maps


def kernel(**inputs):
    global _NC_CACHE
    from concourse.bass_utils import run_bass_kernel_spmd

    if _NC_CACHE is None:
        _NC_CACHE = build_bass()
    nc = _NC_CACHE

    in_maps = make_in_maps(**inputs)
    res = run_bass_kernel_spmd(nc, in_maps, core_ids=list(range(8))).results
    parts = [res[c]["out"].astype(np.float32) for c in range(8)]
    bp = inputs["bp"]
    out = np.stack(
        [
            parts[0] + parts[1] + parts[2] + parts[3] + bp[None, :],
            parts[4] + parts[5] + parts[6] + parts[7] + bp[None, :],
        ]
    )
    return out.astype(np.float32)
